# revision 12
# baseline (speedup 1.0000x reference)
"""Trainium2 Bass kernel for nn_AttentionBlock (GroupNorm + 8-head self-attention
+ out-proj + residual) on [8, 32, 32, 512] inputs.

Sharding: data-parallel over batch — each of the 8 NeuronCores processes one
batch element [1024, 512] end-to-end; weights are replicated. No collectives.

Per-core layout strategy:
  - x [s=1024, c=512] is PE-transposed to xT [c, s]; GroupNorm stats are
    computed per channel with bn_stats, aggregated over the 16 channels of a
    group with tiny indicator matmuls, and the affine normalize writes
    xnT [c, s] in bf16.
  - qT/kT [j, s] come from matmuls with W as stationary; V stays natural
    [s, d] (xnT chunks as stationary). A column of ones is appended to V so
    the attention-value matmul also produces the softmax denominators.
  - Per head: S^T[j, i] = kT.T @ qT (K=64; head pairs land on PE row groups
    0/64 and overlap), exp on ScalarE with the 1/64 scale folded in (logits
    are O(1), so no max subtraction), then attnT[65, i] = V'.T @ exp(S^T)
    accumulated over key chunks. Row 64 holds the denominators.
  - Denominators: DMA-gather to [16, 64], VectorE reciprocal, DMA round-trip
    through DRAM to broadcast across partitions, then one tensor_tensor
    multiply normalizes each head.
  - Out-proj uses attnT as stationary so the result lands natural [s, c'];
    the residual + output bias are folded into the PSUM evacuation.
"""

import numpy as np

B, S, C = 8, 1024, 512
NH, HD = 8, 64
GS = 16            # channels per GroupNorm group
NG = C // GS       # 32 groups
EPS = 1e-6
P = 128
NCT = C // P       # 4 channel tiles
NST = S // P       # 8 seq tiles
NJT = (2 * C) // P # 8 q+k row chunks

_CACHE = {}


def build_nc():
    import concourse.bass as bass
    import concourse.tile as tile
    from concourse import bacc, mybir

    f32 = mybir.dt.float32
    bf16 = mybir.dt.bfloat16
    Alu = mybir.AluOpType
    Act = mybir.ActivationFunctionType

    nc = bacc.Bacc()

    x_d = nc.dram_tensor("x", [S, C], f32, kind="ExternalInput")
    gns_d = nc.dram_tensor("gn_scale", [C], f32, kind="ExternalInput")
    gnb_d = nc.dram_tensor("gn_bias", [C], f32, kind="ExternalInput")
    wqkv_d = nc.dram_tensor("w_qkv", [C, 3 * C], f32, kind="ExternalInput")
    bqkv_d = nc.dram_tensor("b_qkv", [3 * C], f32, kind="ExternalInput")
    wout_d = nc.dram_tensor("w_out", [C, C], f32, kind="ExternalInput")
    bout_d = nc.dram_tensor("b_out", [C], f32, kind="ExternalInput")
    out_d = nc.dram_tensor("out", [S, C], f32, kind="ExternalOutput")

    ident_np = np.eye(P, dtype=np.float32)
    # group indicator: gind[c, g] = 1 if channel c (within a 128-channel tile)
    # belongs to group g (8 groups of 16 per tile)
    gind_np = np.zeros((P, P // GS), dtype=np.float32)
    for c in range(P):
        gind_np[c, c // GS] = 1.0
    ident_d = nc.inline_tensor(ident_np, name="ident")
    gind_d = nc.inline_tensor(gind_np, name="gind")
    gpart_d = nc.inline_tensor(np.ascontiguousarray(gind_np.T), name="gpart")

    with tile.TileContext(nc) as tc:
        with (
            tc.tile_pool(name="consts", bufs=1) as consts,
            tc.tile_pool(name="persist", bufs=1) as persist,
            tc.tile_pool(name="small", bufs=2) as small,
            tc.tile_pool(name="stexp", bufs=3) as stexp_pool,
            tc.tile_pool(name="th", bufs=3) as th_pool,
            tc.tile_pool(name="denb", bufs=2) as denb_pool,
            tc.tile_pool(name="dram", bufs=2, space="DRAM") as dram_pool,
        ):
            # ---------------- constant / weight loads ----------------
            ident = consts.tile([P, P], f32, tag="ident")
            nc.sync.dma_start(out=ident, in_=ident_d[:, :])
            gind = consts.tile([P, P // GS], f32, tag="gind")
            nc.sync.dma_start(out=gind, in_=gind_d[:, :])
            gpart = consts.tile([P // GS, P], f32, tag="gpart")
            nc.sync.dma_start(out=gpart, in_=gpart_d[:, :])

            wqkv_sb = []
            for ct in range(NCT):
                w = consts.tile([P, 3 * C], bf16, tag=f"wqkv{ct}")
                nc.gpsimd.dma_start(out=w, in_=wqkv_d[ct * P:(ct + 1) * P, :])
                wqkv_sb.append(w)
            wout_sb = []
            for ct in range(NCT):
                w = consts.tile([P, C], bf16, tag=f"wout{ct}")
                nc.gpsimd.dma_start(out=w, in_=wout_d[ct * P:(ct + 1) * P, :])
                wout_sb.append(w)

            # per-partition bias columns for the qT/kT chunks
            bqk = consts.tile([P, NJT], f32, tag="bqk")
            nc.sync.dma_start(
                out=bqk, in_=bqkv_d[0:2 * C].rearrange("(jt p) -> p jt", p=P)
            )
            bv_bc = consts.tile([P, C], f32, tag="bv_bc")
            nc.sync.dma_start(
                out=bv_bc,
                in_=bass.AP(tensor=bqkv_d, offset=2 * C, ap=[[0, P], [1, C]]),
            )
            bout_bc = consts.tile([P, C], f32, tag="bout_bc")
            nc.sync.dma_start(
                out=bout_bc, in_=bass.AP(tensor=bout_d, offset=0, ap=[[0, P], [1, C]])
            )
            gnsT = consts.tile([P, NCT], f32, tag="gnsT")
            nc.sync.dma_start(out=gnsT, in_=gns_d.rearrange("(ct p) -> p ct", p=P))
            gnbT = consts.tile([P, NCT], f32, tag="gnbT")
            nc.sync.dma_start(out=gnbT, in_=gnb_d.rearrange("(ct p) -> p ct", p=P))

            eps_sb = consts.tile([P, 1], f32, tag="eps")
            nc.vector.memset(eps_sb, EPS)
            zero_sb = consts.tile([P, 1], f32, tag="zero")
            nc.vector.memset(zero_sb, 0.0)

            x_all = persist.tile([P, NST, C], f32, tag="x_all")
            x_v = x_d.rearrange("(st p) c -> p st c", p=P)
            for st in range(NST):
                nc.sync.dma_start(out=x_all[:, st, :], in_=x_v[:, st, :])

            # ---------------- transpose + GroupNorm ----------------
            xT_sb = persist.tile([P, NCT, S], f32, tag="xT")
            xn_sb = persist.tile([P, NCT, S], bf16, tag="xn")
            prep = persist.tile([P, NCT, 2], f32, tag="prep")

            with tc.tile_pool(name="ps_xt", bufs=2, space="PSUM") as ps_xt, \
                 tc.tile_pool(name="ps_gn", bufs=2, space="PSUM") as ps_gn:
                for ct in range(NCT):
                    xt_ps = ps_xt.tile([P, S], f32, tag="xt")
                    for st in range(NST):
                        nc.tensor.transpose(
                            out=xt_ps[:, st * P:(st + 1) * P],
                            in_=x_all[:, st, ct * P:(ct + 1) * P],
                            identity=ident,
                        )
                    nc.scalar.copy(out=xT_sb[:, ct, :], in_=xt_ps)
                    st6 = small.tile([P, 2, 6], f32, tag="st6")
                    nc.vector.bn_stats(out=st6[:, 0, :], in_=xT_sb[:, ct, 0:512])
                    nc.vector.bn_stats(out=st6[:, 1, :], in_=xT_sb[:, ct, 512:1024])
                    mv = small.tile([P, 2], f32, tag="mv")
                    nc.vector.bn_aggr(out=mv, in_=st6)
                    sq = small.tile([P, 1], f32, tag="sq")
                    nc.vector.tensor_tensor(sq, mv[:, 0:1], mv[:, 0:1], Alu.mult)
                    nc.vector.tensor_tensor(prep[:, ct, 1:2], sq, mv[:, 1:2], Alu.add)
                    nc.vector.tensor_copy(out=prep[:, ct, 0:1], in_=mv[:, 0:1])

                # aggregate (mean, E[x^2]) over the 16 channels of each group,
                # then broadcast back to per-channel columns
                gs_ps = ps_gn.tile([P // GS, 2 * NCT], f32, tag="gs")
                nc.tensor.matmul(gs_ps, lhsT=gind, rhs=prep[:, :, :], start=True, stop=True)
                gs_sb = small.tile([P // GS, 2 * NCT], f32, tag="gs_sb")
                nc.vector.tensor_copy(out=gs_sb, in_=gs_ps)
                gb_ps = ps_gn.tile([P, 2 * NCT], f32, tag="gb")
                nc.tensor.matmul(gb_ps, lhsT=gpart, rhs=gs_sb, start=True, stop=True)

                stats = small.tile([P, 2, NCT], f32, tag="stats")  # [mu, var]
                gb_v = gb_ps.rearrange("p (ct two) -> p ct two", two=2)
                nc.vector.tensor_scalar(
                    out=stats[:, 0, :], in0=gb_v[:, :, 0], scalar1=1.0 / GS,
                    scalar2=None, op0=Alu.mult,
                )
                nc.vector.tensor_scalar(
                    out=stats[:, 1, :], in0=gb_v[:, :, 1], scalar1=1.0 / GS,
                    scalar2=None, op0=Alu.mult,
                )
                musq = small.tile([P, NCT], f32, tag="musq")
                nc.vector.tensor_tensor(musq, stats[:, 0, :], stats[:, 0, :], Alu.mult)
                var = small.tile([P, NCT], f32, tag="var")
                nc.vector.tensor_tensor(var, stats[:, 1, :], musq, Alu.subtract)
                sd = small.tile([P, NCT], f32, tag="sd")
                nc.scalar.activation(out=sd, in_=var, func=Act.Sqrt, bias=eps_sb)
                rr = small.tile([P, NCT], f32, tag="rr")
                nc.vector.reciprocal(out=rr, in_=sd)
                aa = small.tile([P, NCT], f32, tag="aa")
                nc.vector.tensor_tensor(aa, rr, gnsT, Alu.mult)
                ma = small.tile([P, NCT], f32, tag="ma")
                nc.vector.tensor_tensor(ma, stats[:, 0, :], aa, Alu.mult)
                cc = small.tile([P, NCT], f32, tag="cc")
                nc.vector.tensor_tensor(cc, gnbT, ma, Alu.subtract)

                for ct in range(NCT):
                    nc.vector.tensor_scalar(
                        out=xn_sb[:, ct, :], in0=xT_sb[:, ct, :],
                        scalar1=aa[:, ct:ct + 1], scalar2=cc[:, ct:ct + 1],
                        op0=Alu.mult, op1=Alu.add,
                    )

            # ---------------- QKV projections ----------------
            qk_sb = persist.tile([P, NJT, S], bf16, tag="qk")
            vp_tiles = []
            with tc.tile_pool(name="ps_qkv", bufs=3, space="PSUM") as ps_qkv:
                for jt in range(NJT):
                    for sc in range(2):
                        ps = ps_qkv.tile([P, 512], f32, tag="qkv")
                        for ct in range(NCT):
                            nc.tensor.matmul(
                                ps,
                                lhsT=wqkv_sb[ct][:, jt * P:(jt + 1) * P],
                                rhs=xn_sb[:, ct, sc * 512:(sc + 1) * 512],
                                start=(ct == 0), stop=(ct == NCT - 1),
                            )
                        nc.vector.tensor_scalar(
                            out=qk_sb[:, jt, sc * 512:(sc + 1) * 512], in0=ps,
                            scalar1=bqk[:, jt:jt + 1], scalar2=None, op0=Alu.add,
                        )
                for st in range(NST):
                    ps = ps_qkv.tile([P, 512], f32, tag="qkv")
                    for ct in range(NCT):
                        nc.tensor.matmul(
                            ps,
                            lhsT=xn_sb[:, ct, st * P:(st + 1) * P],
                            rhs=wqkv_sb[ct][:, 2 * C:3 * C],
                            start=(ct == 0), stop=(ct == NCT - 1),
                        )
                    # trailing ones column -> AV matmul row 64 = softmax denoms
                    vp = persist.tile([P, NH, HD + 1], bf16, tag=f"vp{st}")
                    nc.gpsimd.memset(vp[:, :, HD:HD + 1], 1.0)
                    nc.vector.tensor_tensor(
                        vp[:, :, 0:HD],
                        ps.rearrange("p (h d) -> p h d", h=NH),
                        bv_bc.rearrange("p (h d) -> p h d", h=NH),
                        Alu.add,
                    )
                    vp_tiles.append(vp)

            # residual + output bias staged for the out-proj evacuation
            xb_all = persist.tile([P, NST, C], f32, tag="xb")
            for st in range(NST):
                nc.vector.tensor_tensor(
                    xb_all[:, st, :], x_all[:, st, :], bout_bc, Alu.add
                )

            # ---------------- attention ----------------
            atn_sb = persist.tile([P, NCT, S], bf16, tag="atn")
            with tc.tile_pool(name="ps_st", bufs=2, space="PSUM") as ps_st, \
                 tc.tile_pool(name="ps_av", bufs=2, space="PSUM") as ps_av:
                for h in range(NH):
                    qt = h // 2
                    prt = HD * (h % 2)
                    q_ap = qk_sb[prt:prt + HD, qt, :]
                    k_ap = qk_sb[prt:prt + HD, NCT + qt, :]
                    av_ps = ps_av.tile([HD + 1, S], f32, tag="av")
                    se_tiles = {}
                    for jc in range(NST):
                        st_ps = ps_st.tile([P, S], f32, tag="st")
                        for sc in range(2):
                            nc.tensor.matmul(
                                st_ps[:, sc * 512:(sc + 1) * 512],
                                lhsT=k_ap[:, jc * P:(jc + 1) * P],
                                rhs=q_ap[:, sc * 512:(sc + 1) * 512],
                                start=True, stop=True,
                            )
                        se = stexp_pool.tile([P, S], bf16, tag="se")
                        nc.scalar.activation(
                            out=se, in_=st_ps, func=Act.Exp, scale=1.0 / HD,
                            bias=zero_sb,
                        )
                        se_tiles[jc] = se
                        for sc in range(2):
                            nc.tensor.matmul(
                                av_ps[:, sc * 512:(sc + 1) * 512],
                                lhsT=vp_tiles[jc][:, h, :],
                                rhs=se[:, sc * 512:(sc + 1) * 512],
                                start=(jc == 0), stop=(jc == NST - 1),
                            )
                    th = th_pool.tile([HD + 1, S], bf16, tag="th")
                    nc.vector.tensor_copy(out=th, in_=av_ps)
                    # denominators: row 64 -> DRAM -> [16, 64] for a cheap
                    # reciprocal -> DRAM -> partition-broadcast load
                    dscr_a = dram_pool.tile([S], bf16, tag="dscr_a")
                    nc.sync.dma_start(out=dscr_a, in_=th[HD:HD + 1, :])
                    r0 = small.tile([16, HD], bf16, tag="r0")
                    nc.sync.dma_start(
                        out=r0, in_=dscr_a.rearrange("(p c) -> p c", p=16)
                    )
                    r1 = small.tile([16, HD], bf16, tag="r1")
                    with nc.allow_low_precision("softmax denominators in bf16"):
                        nc.vector.reciprocal(out=r1, in_=r0)
                    dscr_b = dram_pool.tile([S], bf16, tag="dscr_b")
                    nc.sync.dma_start(out=dscr_b, in_=r1)
                    denb = denb_pool.tile([HD, S], bf16, tag="denb")
                    nc.sync.dma_start(
                        out=denb,
                        in_=bass.AP(tensor=dscr_b.tensor, offset=dscr_b.offset,
                                    ap=[[0, HD]] + dscr_b.ap),
                    )
                    nc.vector.tensor_tensor(
                        atn_sb[prt:prt + HD, qt, :], th[0:HD, :], denb, Alu.mult
                    )

            # ---------------- out-proj + residual ----------------
            with tc.tile_pool(name="ps_pj", bufs=2, space="PSUM") as ps_pj:
                for st in range(NST):
                    ps = ps_pj.tile([P, C], f32, tag="pj")
                    for pt in range(NCT):
                        nc.tensor.matmul(
                            ps,
                            lhsT=atn_sb[:, pt, st * P:(st + 1) * P],
                            rhs=wout_sb[pt],
                            start=(pt == 0), stop=(pt == NCT - 1),
                        )
                    ot = small.tile([P, C], f32, tag="ot")
                    nc.vector.tensor_tensor(ot, ps, xb_all[:, st, :], Alu.add)
                    nc.sync.dma_start(
                        out=out_d.rearrange("(st p) c -> p st c", p=P)[:, st, :],
                        in_=ot,
                    )
    nc.compile()
    return nc


def _get_nc():
    if "nc" not in _CACHE:
        _CACHE["nc"] = build_nc()
    return _CACHE["nc"]


def kernel(x, gn_scale, gn_bias, w_qkv, b_qkv, w_out, b_out):
    from concourse.bass_utils import run_bass_kernel_spmd

    nc = _get_nc()
    x = np.asarray(x, dtype=np.float32)
    b, h, w, c = x.shape
    shared = {
        "gn_scale": np.ascontiguousarray(np.asarray(gn_scale, np.float32)),
        "gn_bias": np.ascontiguousarray(np.asarray(gn_bias, np.float32)),
        "w_qkv": np.ascontiguousarray(np.asarray(w_qkv, np.float32)),
        "b_qkv": np.ascontiguousarray(np.asarray(b_qkv, np.float32)),
        "w_out": np.ascontiguousarray(np.asarray(w_out, np.float32)),
        "b_out": np.ascontiguousarray(np.asarray(b_out, np.float32)),
    }
    in_maps = [
        {"x": np.ascontiguousarray(x[i].reshape(S, C)), **shared} for i in range(B)
    ]
    res = run_bass_kernel_spmd(nc, in_maps, core_ids=list(range(B)))
    out = np.stack([r["out"] for r in res.results], axis=0)
    return out.reshape(b, h, w, c).astype(np.float32)
